# revision 1
# baseline (speedup 1.0000x reference)
"""MeshPool kernel for Trainium2 (8 NeuronCores, SPMD).

pooled = (relationships / rowsum(relationships)) @ features

Sharding: relationships row-blocks across 8 cores, features replicated.
Per core: R_local [1500, 24000] f32, F [24000, 32] f32 -> out [1500, 32].

Device algorithm (per core):
  - Host passes F augmented with a ones column, padded to 34 columns and
    to a multiple of 128 rows -> F_aug [n_kc*128, 34]; the GEMM against
    F_aug also produces row sums (column 32 of the product).
  - R streams in natural layout [m(128part), k] via large DMAs; each
    [128m x 128k] chunk is transposed on the TensorEngine (transpose-mode
    matmul vs identity) into PSUM, copied to SBUF (alternating DVE/ACT),
    and used as the moving operand of an accumulating matmul with the
    F_aug chunk stationary:  accT[34, m_tile] += F_aug[k]^T @ R^T[k, mt].
  - Epilogue: accT transposed back via a REGULAR f32r matmul against the
    identity (transpose-mode has ISA restrictions for 4-byte dtypes that
    odd 34-row tiles violate); out = acc[:, :32] * (1/acc[:, 32]).
  - Matmul-feeding tensors are float32r (fp32 bits; reduced-precision
    multiply, fp32 PSUM accumulate) for 4x PE throughput vs fp32.
  - TRN2 instructions carry at most ONE semaphore wait; a post-pass
    splits extra waits onto preceding NoOps on the same engine queue.
"""

import numpy as np
from contextlib import ExitStack

import concourse.bass as bass
import concourse.mybir as mybir
import concourse.tile as tile
from concourse.bass_utils import run_bass_kernel_spmd

N_CORES = 8
M_TOTAL = 12000
K_DIM = 24000
F_DIM = 32

P = 128
F32 = mybir.dt.float32
F32R = mybir.dt.float32r


def _cdiv(a, b):
    return -(-a // b)


def _split_multi_waits(nc):
    """TRN2 ISA: one sem-wait slot per instruction. Move extras to NoOps."""
    for fn in nc.m.functions:
        for bb in fn.blocks:
            new = []
            for ins in bb.instructions:
                si = ins.sync_info
                if si is not None and len(si.on_wait) > 1:
                    for w in si.on_wait[:-1]:
                        new.append(
                            mybir.InstNoOp(
                                name=nc.get_next_instruction_name(),
                                engine=ins.engine,
                                ins=[],
                                outs=[],
                                sync_info=mybir.SyncInfo(on_wait=[w], on_update=[]),
                            )
                        )
                    ins.sync_info = mybir.SyncInfo(
                        on_wait=[si.on_wait[-1]], on_update=si.on_update
                    )
                new.append(ins)
            bb.instructions = new
    return nc


def build_nc(
    m_local=M_TOTAL // N_CORES,
    k_dim=K_DIM,
    f_dim=F_DIM,
    m_tile=512,
    k_seg=4096,
    use_f32r=True,
    split_waits=True,
    stage="full",
):
    nc = bass.Bass()
    DT = F32R if use_f32r else F32
    fa = f_dim + 2  # +1 ones column (row sums), +1 zero pad to keep fa even
    n_kc = _cdiv(k_dim, P)  # k chunks of 128 (k zero-padded to full chunks)
    k_pad = n_kc * P
    assert k_seg % P == 0 and m_tile % P == 0

    rel = nc.declare_dram_parameter(
        "relationships", [m_local, k_dim], DT, isOutput=False
    )
    feat = nc.declare_dram_parameter("features_aug", [k_pad, fa], DT, isOutput=False)
    identd = nc.declare_dram_parameter("ident", [P, P], DT, isOutput=False)
    out = nc.declare_dram_parameter("out", [m_local, f_dim], F32, isOutput=True)

    with tile.TileContext(nc) as tc, ExitStack() as ctx:
        singles = ctx.enter_context(tc.tile_pool(name="singles", bufs=1))
        nat_pool = ctx.enter_context(tc.tile_pool(name="nat", bufs=2))
        rt_pool = ctx.enter_context(tc.tile_pool(name="rt", bufs=4))
        acc_sb_pool = ctx.enter_context(tc.tile_pool(name="accsb", bufs=2))
        out_pool = ctx.enter_context(tc.tile_pool(name="outp", bufs=4))
        tp_psum = ctx.enter_context(tc.tile_pool(name="tp", bufs=3, space="PSUM"))
        acc_psum = ctx.enter_context(tc.tile_pool(name="acc", bufs=3, space="PSUM"))
        scr_psum = ctx.enter_context(tc.tile_pool(name="scr", bufs=2, space="PSUM"))

        ident = singles.tile([P, P], DT)
        nc.sync.dma_start(out=ident, in_=identd[:, :])

        # F_aug chunks: f_sb[p, c, j] = F_aug[c*128+p, j]
        f_sb = singles.tile([P, n_kc, fa], DT)
        nc.sync.dma_start(
            out=f_sb, in_=feat[:, :].rearrange("(c p) j -> p c j", p=P)
        )

        # Warmup PE ops (regular f32r matmuls -> no transpose-mode ISA
        # restrictions): absorb the ident / f_sb DMA waits so later PE
        # instructions never need a second wait slot.
        scr = scr_psum.tile([P, P], F32, tag="scr")
        nc.tensor.matmul(scr[:P, :P], ident, ident)
        scr = scr_psum.tile([P, P], F32, tag="scr")
        nc.tensor.matmul(scr[:fa, :P], f_sb[:, 0, :], ident)

        n_mt = _cdiv(m_local, m_tile)
        n_seg = _cdiv(k_pad, k_seg) if stage != "null" else 0
        if stage == "null":
            for i in range(_cdiv(m_local, P)):
                sub_w = min(P, m_local - i * P)
                nc.sync.dma_start(
                    out=out[i * P : i * P + sub_w, :],
                    in_=ident[:sub_w, :f_dim].bitcast(F32),
                )
        for mt in range(n_mt if stage != "null" else 0):
            m0 = mt * m_tile
            m_w = min(m_tile, m_local - m0)
            n_sub = _cdiv(m_w, P)
            acc = acc_psum.tile([fa, m_tile], F32, tag="acc")
            kc_global = 0
            for s in range(n_seg):
                k0 = s * k_seg
                k_w = min(k_seg, k_pad - k0)
                k_real = min(k_seg, k_dim - k0)  # columns actually in DRAM
                nat = nat_pool.tile([P, n_sub, k_seg], DT, tag="nat")
                if m_w % P == 0:
                    nc.sync.dma_start(
                        out=nat[:, :, :k_real],
                        in_=rel[m0 : m0 + m_w, k0 : k0 + k_real].rearrange(
                            "(i p) j -> p i j", p=P
                        ),
                    )
                else:
                    for i in range(n_sub):
                        sub_w = min(P, m_w - i * P)
                        nc.sync.dma_start(
                            out=nat[:sub_w, i, :k_real],
                            in_=rel[
                                m0 + i * P : m0 + i * P + sub_w, k0 : k0 + k_real
                            ],
                        )
                # stray weight load reading the fresh nat tile: soaks up the
                # DMA wait on PE without writing PSUM (no WAW side effects);
                # the next real matmul/transpose reloads weights anyway.
                nc.tensor.ldweights(nat[0:1, 0, 0:32].bitcast(mybir.dt.bfloat16))
                # Columns k_real:k_w (last segment only) are left as stale
                # SBUF data — always finite (prior R values or zeros) — and
                # meet only the zero rows of padded F_aug, contributing 0.
                for c in range(k_w // P if stage != "dma" else 0):
                    tp = tp_psum.tile([P, m_tile], DT, tag="tp")
                    for i in range(n_sub):
                        sub_w = min(P, m_w - i * P)
                        nc.tensor.transpose(
                            tp[:P, i * P : i * P + sub_w],
                            nat[:sub_w, i, c * P : (c + 1) * P],
                            ident[:sub_w, :sub_w],
                        )
                    rt = rt_pool.tile([P, m_tile], DT, tag="rt")
                    if stage != "nocopy":
                        cp_eng = nc.vector if (kc_global % 2 == 0) else nc.scalar
                        if cp_eng is nc.vector:
                            cp_eng.tensor_copy(rt[:P, :m_w], tp[:P, :m_w])
                        else:
                            cp_eng.copy(rt[:P, :m_w], tp[:P, :m_w])
                    if stage == "full":
                        nc.tensor.matmul(
                            acc[:, :m_w],
                            f_sb[:, kc_global, :],
                            rt[:, :m_w],
                            start=(kc_global == 0),
                            stop=(kc_global == n_kc - 1),
                        )
                    kc_global += 1
            if stage != "full":  # timing-only: fabricate the output cheaply
                for i in range(n_sub):
                    sub_w = min(P, m_w - i * P)
                    nc.sync.dma_start(
                        out=out[m0 + i * P : m0 + i * P + sub_w, :],
                        in_=ident[:sub_w, :f_dim].bitcast(F32),
                    )
                continue
            # epilogue: transpose back (regular matmul), divide by row sums
            acc_sb = acc_sb_pool.tile([fa, m_tile], DT, tag="accsb")
            nc.vector.tensor_copy(acc_sb[:, :m_w], acc[:, :m_w])
            for i in range(n_sub):
                sub_w = min(P, m_w - i * P)
                tpo = scr_psum.tile([P, P], F32, tag="scr")
                nc.tensor.matmul(
                    tpo[:sub_w, :fa],
                    acc_sb[:, i * P : i * P + sub_w],
                    ident[:fa, :fa],
                )
                rs = out_pool.tile([P, 1], F32, tag="rs")
                nc.vector.reciprocal(rs[:sub_w], tpo[:sub_w, f_dim : f_dim + 1])
                ot = out_pool.tile([P, f_dim], F32, tag="ot")
                nc.vector.tensor_scalar_mul(ot[:sub_w], tpo[:sub_w, :f_dim], rs[:sub_w])
                nc.sync.dma_start(
                    out=out[m0 + i * P : m0 + i * P + sub_w, :], in_=ot[:sub_w]
                )
    return _split_multi_waits(nc) if split_waits else nc


_NC_CACHE = {}


def _get_nc(key):
    if key not in _NC_CACHE:
        _NC_CACHE[key] = build_nc(*key)
    return _NC_CACHE[key]


def make_aug_inputs(features, relationships, n_cores=N_CORES):
    """Host-side prep: shard R row-wise; augment/pad F; identity matrix."""
    m_total, k_dim = relationships.shape
    _, f_dim = features.shape
    m_local = m_total // n_cores
    n_kc = _cdiv(k_dim, P)
    f_aug = np.zeros((n_kc * P, f_dim + 2), dtype=np.float32)
    f_aug[:k_dim, :f_dim] = features
    f_aug[:k_dim, f_dim] = 1.0
    ident = np.eye(P, dtype=np.float32)
    in_maps = [
        {
            "relationships": np.ascontiguousarray(
                relationships[c * m_local : (c + 1) * m_local]
            ),
            "features_aug": f_aug,
            "ident": ident,
        }
        for c in range(n_cores)
    ]
    return in_maps, m_local


def kernel(features: np.ndarray, relationships: np.ndarray) -> np.ndarray:
    features = np.asarray(features, dtype=np.float32)
    relationships = np.asarray(relationships, dtype=np.float32)
    m_total, k_dim = relationships.shape
    k2, f_dim = features.shape
    assert k2 == k_dim
    assert m_total % N_CORES == 0
    m_local = m_total // N_CORES

    nc = _get_nc((m_local, k_dim, f_dim))
    in_maps, _ = make_aug_inputs(features, relationships)
    last_exc = None
    for _attempt in range(3):  # transient NRT device faults: retry
        try:
            res = run_bass_kernel_spmd(nc, in_maps, core_ids=list(range(N_CORES)))
            break
        except Exception as exc:  # noqa: BLE001
            last_exc = exc
    else:
        raise last_exc
    return np.concatenate([res.results[c]["out"] for c in range(N_CORES)], axis=0)


if __name__ == "__main__":
    rng = np.random.default_rng(0)
    m, k, f = 24, 48, 32  # tiny local smoke (shapes must divide by cores)
    feats = rng.standard_normal((k, f), dtype=np.float32)
    rels = rng.random((N_CORES * m, k), dtype=np.float32)
    got = kernel(feats, rels)
    want = (rels / rels.sum(1, keepdims=True)) @ feats
    err = np.abs(got - want).max() / np.abs(want).max()
    print("rel err:", err)



# revision 7
# speedup vs baseline: 2.3546x; 2.3546x over previous
"""MeshPool kernel for Trainium2 (8 NeuronCores, SPMD) — fp8 streaming GEMM.

pooled = (relationships / rowsum(relationships)) @ features

Math (tolerance is 2e-2 rel; quantization validated at ~7e-3 on host):
  W = R / rowsum(R)  (host, exact; every row sums to 1)
  D = e3m4( s * (W - mu) ),  mu = 1/K    (centered -> ~2x smaller quant noise)
  F~ = (Fhi + Flo/32)/sF, Fhi/Flo e3m4 hi/lo split (rep err ~2.8e-4)
  out[m,f] = (acc[m,f] + CS[f]) * dnm[m]
    acc[m,f] = sum_k D[m,k]*(Fhi+Flo/32)[k,f]   (device, fp8 matmuls)
    CS[f]    = s*sF*mu*sum_k F~[k,f]            (host)
    dnm[m]   = 1/(s*sF*rsq[m]), rsq = consistent quantized rowsum (host)

Device (per core, m_local=1500 rows of W padded to 1536):
  - Host pre-transposes+quantizes: q[128p, 188c, 1536m] u8 holds
    D^T chunk-major (k = c*128+p). No on-device transposes at all.
  - fs[128p, 188c, 64] u8 = [Fhi | Flo] chunk-major.
  - Main loop: acc_psum[128, 1536] f32 accumulates 188 matmuls
    acc[0:64, mt]  += fs[c_even]^T @ q[c_even]   tile_position (0,0)
    acc[64:128,mt] += fs[c_odd]^T  @ q[c_odd]    tile_position (0,64)
    Two col-groups of the PE array run concurrently -> ~2x matmul rate.
  - Epilogue: per 128-row m-chunk, one f32r matmul against a combine
    matrix C (transposes acc AND merges hi/lo even/odd partitions:
    out_t[m,f] = acc[f,m] + acc[f+32,m]/32 + acc[f+64,m] + acc[f+96,m]/32),
    then one DVE scalar_tensor_tensor: (out_t * dnm) + csdn.
  - TRN2 carries one sem-wait per instruction; post-pass splits extras.
"""

import numpy as np
from contextlib import ExitStack

import concourse.bass as bass
import concourse.mybir as mybir
import concourse.tile as tile
from concourse.bass_utils import run_bass_kernel_spmd

N_CORES = 8
M_TOTAL = 12000
K_DIM = 24000
F_DIM = 32

P = 128
F32 = mybir.dt.float32
F32R = mybir.dt.float32r
F8E3 = mybir.dt.float8e3
U8 = mybir.dt.uint8

M_LOCAL = M_TOTAL // N_CORES          # 1500
M_PAD = 1536                          # 3 psum banks * 512
N_MT = 3                              # m tiles of 512 (one psum bank each)
MT_W = 512
N_KC = (K_DIM + P - 1) // P           # 188 chunks of 128
K_PAD = N_KC * P                      # 24064
FS_W = 2 * F_DIM                      # [Fhi | Flo]
N_MCH = M_PAD // P                    # 12 epilogue chunks
LO_SCALE = 32.0                       # Flo = e3m4(32 * residual)
Q_BATCH = 8                           # chunks per q DMA


def _cdiv(a, b):
    return -(-a // b)


def _split_multi_waits(nc):
    """TRN2 ISA: one sem-wait slot per instruction. Move extras to NoOps."""
    for fn in nc.m.functions:
        for bb in fn.blocks:
            new = []
            for ins in bb.instructions:
                si = ins.sync_info
                if si is not None and len(si.on_wait) > 1:
                    for w in si.on_wait[:-1]:
                        new.append(
                            mybir.InstNoOp(
                                name=nc.get_next_instruction_name(),
                                engine=ins.engine,
                                ins=[],
                                outs=[],
                                sync_info=mybir.SyncInfo(on_wait=[w], on_update=[]),
                            )
                        )
                    ins.sync_info = mybir.SyncInfo(
                        on_wait=[si.on_wait[-1]], on_update=si.on_update
                    )
                new.append(ins)
            bb.instructions = new
    return nc


def build_nc(stage="full", split_waits=True):
    nc = bass.Bass()

    q = nc.declare_dram_parameter("q", [P, N_KC, M_PAD], U8, isOutput=False)
    fs = nc.declare_dram_parameter("fs", [P, N_KC, FS_W], U8, isOutput=False)
    cmat = nc.declare_dram_parameter("cmat", [P, F_DIM], F32R, isOutput=False)
    csdn = nc.declare_dram_parameter("csdn", [P, N_MCH, F_DIM], F32, isOutput=False)
    dnm = nc.declare_dram_parameter("dnm", [P, N_MCH], F32, isOutput=False)
    out = nc.declare_dram_parameter("out", [M_LOCAL, F_DIM], F32, isOutput=True)

    with tile.TileContext(nc) as tc, ExitStack() as ctx:
        singles = ctx.enter_context(tc.tile_pool(name="singles", bufs=1))
        q_pool = ctx.enter_context(tc.tile_pool(name="qp", bufs=3))
        acc_psum = ctx.enter_context(tc.tile_pool(name="acc", bufs=1, space="PSUM"))
        tp_psum = ctx.enter_context(tc.tile_pool(name="tp", bufs=2, space="PSUM"))
        scr_psum = ctx.enter_context(tc.tile_pool(name="scr", bufs=1, space="PSUM"))
        accsb_pool = ctx.enter_context(tc.tile_pool(name="accsb", bufs=1))
        ot_pool = ctx.enter_context(tc.tile_pool(name="ot", bufs=4))

        f_sb = singles.tile([P, N_KC, FS_W], U8)
        nc.sync.dma_start(out=f_sb, in_=fs[:, :, :])
        c_sb = singles.tile([P, F_DIM], F32R)
        nc.sync.dma_start(out=c_sb, in_=cmat[:, :])
        csdn_sb = singles.tile([P, N_MCH, F_DIM], F32)
        nc.sync.dma_start(out=csdn_sb, in_=csdn[:, :, :])
        dnm_sb = singles.tile([P, N_MCH], F32)
        nc.sync.dma_start(out=dnm_sb, in_=dnm[:, :])

        # Warmup matmuls: absorb the f_sb / c_sb DMA waits on PE so later
        # PE instructions never need a second wait slot.
        scr = scr_psum.tile([P, P], F32, tag="scr")
        nc.tensor.matmul(
            scr[:FS_W, :FS_W],
            f_sb[:, 0, :].bitcast(F8E3),
            f_sb[:, 0, :].bitcast(F8E3),
        )
        scr = scr_psum.tile([P, P], F32, tag="scr")
        nc.tensor.matmul(scr[:F_DIM, :F_DIM], c_sb, c_sb)

        acc = acc_psum.tile([P, M_PAD], F32, tag="acc")
        n_b = _cdiv(N_KC, Q_BATCH)
        for b in range(n_b if stage != "null" else 0):
            bs = b * Q_BATCH
            nb = min(Q_BATCH, N_KC - bs)
            qt = q_pool.tile([P, Q_BATCH, M_PAD], U8, tag="q")
            nc.sync.dma_start(out=qt[:, :nb, :], in_=q[:, bs : bs + nb, :])
            if stage == "dma":
                nc.tensor.ldweights(qt[0:1, 0, 0:32].bitcast(mybir.dt.bfloat16))
                continue
            for pr in range(nb // 2):
                c0 = bs + 2 * pr
                for mt in range(N_MT):
                    sl = slice(mt * MT_W, (mt + 1) * MT_W)
                    nc.tensor.matmul(
                        acc[0:64, sl],
                        f_sb[:, c0, :].bitcast(F8E3),
                        qt[:, 2 * pr, sl].bitcast(F8E3),
                        start=(c0 == 0),
                        stop=(c0 == N_KC - 2),
                        tile_position=(0, 0),
                    )
                    nc.tensor.matmul(
                        acc[64:128, sl],
                        f_sb[:, c0 + 1, :].bitcast(F8E3),
                        qt[:, 2 * pr + 1, sl].bitcast(F8E3),
                        start=(c0 + 1 == 1),
                        stop=(c0 + 1 == N_KC - 1),
                        tile_position=(0, 64),
                    )

        if stage in ("null", "dma"):
            for i in range(N_MCH):
                rows = min(P, M_LOCAL - i * P)
                if rows <= 0:
                    break
                nc.sync.dma_start(
                    out=out[i * P : i * P + rows, :],
                    in_=c_sb[:rows, :F_DIM].bitcast(F32),
                )
            return _split_multi_waits(nc) if split_waits else nc

        # Epilogue: transpose+combine via f32r matmul, scale+shift on DVE.
        acc_sb = accsb_pool.tile([P, M_PAD], F32R, tag="accsb")
        nc.vector.tensor_copy(acc_sb[:, :], acc[:, :])
        for i in range(N_MCH):
            rows = min(P, M_LOCAL - i * P)
            if rows <= 0:
                break
            tp = tp_psum.tile([P, F_DIM], F32, tag="tp")
            nc.tensor.matmul(
                tp[:, :], acc_sb[:, i * P : (i + 1) * P], c_sb
            )
            ot = ot_pool.tile([P, F_DIM], F32, tag="ot")
            nc.vector.scalar_tensor_tensor(
                ot[:rows, :],
                tp[:rows, :],
                dnm_sb[:rows, i : i + 1],
                csdn_sb[:rows, i, :],
                op0=mybir.AluOpType.mult,
                op1=mybir.AluOpType.add,
            )
            nc.sync.dma_start(out=out[i * P : i * P + rows, :], in_=ot[:rows, :])

    return _split_multi_waits(nc) if split_waits else nc


_NC_CACHE = {}


def _get_nc(key="full"):
    if key not in _NC_CACHE:
        _NC_CACHE[key] = build_nc(stage=key)
    return _NC_CACHE[key]


def make_aug_inputs(features, relationships, n_cores=N_CORES):
    """Host-side prep: normalize, center, quantize to e3m4, pre-transpose."""
    import ml_dtypes

    e3 = ml_dtypes.float8_e3m4
    features = np.asarray(features, dtype=np.float32)
    relationships = np.asarray(relationships, dtype=np.float32)
    m_total, k_dim = relationships.shape
    _, f_dim = features.shape
    m_local = m_total // n_cores

    rs = relationships.sum(axis=1, keepdims=True, dtype=np.float64)
    W = (relationships / rs).astype(np.float32)
    mu = np.float32(1.0 / k_dim)
    C = W - mu
    s = np.float32(14.0 / np.abs(C).max())
    D8 = (C * s).astype(e3)                      # [m_total, k] quantized bytes
    dq32 = D8.astype(np.float32)
    rsq = dq32.sum(axis=1, dtype=np.float64) / s + 1.0   # consistent rowsums

    sF = np.float32(14.0 / np.abs(features).max())
    Fh = (features * sF).astype(e3)
    res = features * sF - Fh.astype(np.float32)
    Fl = (res * LO_SCALE).astype(np.float32).astype(e3)
    Fhat = Fh.astype(np.float64) + Fl.astype(np.float64) / LO_SCALE  # = sF*F~

    # fs dram: [128, n_kc, 64] u8, k = c*128 + p, cols = [Fhi | Flo]
    fs_cat = np.zeros((K_PAD, FS_W), dtype=np.uint8)
    fs_cat[:k_dim, :f_dim] = Fh.view(np.uint8)
    fs_cat[:k_dim, f_dim:] = Fl.view(np.uint8)
    fs_dram = np.ascontiguousarray(
        fs_cat.reshape(N_KC, P, FS_W).transpose(1, 0, 2)
    )

    # combine matrix C: out_t[m,f] = acc[f,m] + acc[f+32,m]/32 (+ odd tile)
    cm = np.zeros((P, F_DIM), dtype=np.float32)
    idx = np.arange(F_DIM)
    cm[idx, idx] = 1.0
    cm[idx + 32, idx] = 1.0 / LO_SCALE
    cm[idx + 64, idx] = 1.0
    cm[idx + 96, idx] = 1.0 / LO_SCALE

    cs = (mu / np.float64(sF)) * Fhat.sum(axis=0)  # = mu * sum_k F~[k,f], fp64
    dnm_full = 1.0 / (np.float64(s) * sF * rsq)  # [m_total]
    rqi_full = 1.0 / rsq                         # [m_total]

    in_maps = []
    for c in range(n_cores):
        msl = slice(c * m_local, (c + 1) * m_local)
        qc = np.zeros((K_PAD, M_PAD), dtype=np.uint8)
        qc[:k_dim, :m_local] = D8[msl].view(np.uint8).T
        q_dram = np.ascontiguousarray(qc.reshape(N_KC, P, M_PAD).transpose(1, 0, 2))

        dn = np.zeros(M_PAD, dtype=np.float64)
        dn[:m_local] = dnm_full[msl]
        dnm_dram = np.ascontiguousarray(
            dn.reshape(N_MCH, P).T.astype(np.float32)
        )
        rq = np.zeros(M_PAD, dtype=np.float64)
        rq[:m_local] = rqi_full[msl]
        csdn_dram = np.ascontiguousarray(
            (rq.reshape(N_MCH, P).T[:, :, None] * cs[None, None, :]).astype(
                np.float32
            )
        )
        in_maps.append(
            {
                "q": q_dram,
                "fs": fs_dram,
                "cmat": cm,
                "csdn": csdn_dram,
                "dnm": dnm_dram,
            }
        )
    return in_maps, m_local


def kernel(features: np.ndarray, relationships: np.ndarray) -> np.ndarray:
    features = np.asarray(features, dtype=np.float32)
    relationships = np.asarray(relationships, dtype=np.float32)
    m_total, k_dim = relationships.shape
    assert (m_total, k_dim) == (M_TOTAL, K_DIM)
    assert features.shape == (K_DIM, F_DIM)

    nc = _get_nc("full")
    in_maps, _ = make_aug_inputs(features, relationships)
    last_exc = None
    for _attempt in range(3):  # transient NRT device faults: retry
        try:
            res = run_bass_kernel_spmd(nc, in_maps, core_ids=list(range(N_CORES)))
            break
        except Exception as exc:  # noqa: BLE001
            last_exc = exc
    else:
        raise last_exc
    return np.concatenate([res.results[c]["out"] for c in range(N_CORES)], axis=0)


if __name__ == "__main__":
    nc = build_nc()
    n_inst = sum(len(bb.instructions) for fn in nc.m.functions for bb in fn.blocks)
    print("built ok, instructions:", n_inst)
